# revision 2
# baseline (speedup 1.0000x reference)
"""Trainium2 Bass kernel for softmax(relu(nodevec1 @ nodevec2), axis=1).

nodevec1: [8192, 10] f32, nodevec2: [10, 8192] f32 -> out [8192, 8192] f32.

Strategy (8 NeuronCores, no collectives needed):
- Row-shard nodevec1: core i computes rows [i*1024, (i+1)*1024).
- Host-side prep: split each f32 input into bf16 hi+lo pairs and stack
  along the contraction dim (K=30: h1*h2 + l1*h2 + h1*l2), so the PE runs
  at bf16 speed with ~f32 accuracy. Also pre-transpose the nodevec1 shard
  to the [K, M] layout the PE wants for the stationary operand.
- The K=30 operands are loaded twice (SBUF partition offsets 0 and 64) so
  matmuls alternate between two PE row-groups and run pairwise-concurrent
  (tile_position row packing).
- Per 128-row tile: matmul (K=30) -> PSUM, ACT exp: PSUM -> SBUF bf16,
  DVE tensor_scalar max(e,1) with fused row-sum (exp(relu(x)) ==
  max(exp(x),1)), DVE reciprocal, DVE tensor_scalar scale -> bf16 out,
  DMA out. Row softmax is local to each core.
- Output is written bf16 (halves the HBM write) and widened to f32 on the
  host; softmax values are well inside bf16's safe range.
"""

import os

import numpy as np
import ml_dtypes

NODES = 8192
RANK = 10
N_CORES = 8
ROWS_PER_CORE = NODES // N_CORES  # 1024
RT = 128  # rows per tile (SBUF partition dim)
N_RT = ROWS_PER_CORE // RT  # 8
KS = 3 * RANK  # 30: [h1; l1; h1] x [h2; h2; l2]
PSUM_COLS = 2048  # 4 banks per psum tile
MM_N = 512  # one PSUM bank per matmul
GRP = 64  # partition offset of the second PE row-group replica

_cached_nc = None
LAST_RESULTS = None  # BassKernelResults from the most recent run (for test.py)


def _build():
    import concourse.bass as bass
    import concourse.tile as tile
    from concourse import bacc, mybir

    bf16 = mybir.dt.bfloat16
    f32 = mybir.dt.float32
    AF = mybir.ActivationFunctionType
    OP = mybir.AluOpType

    nc = bacc.Bacc(None, target_bir_lowering=False, debug=False)

    n1s = nc.declare_dram_parameter("n1s", [KS, ROWS_PER_CORE], bf16, isOutput=False)
    n2s = nc.declare_dram_parameter("n2s", [KS, NODES], bf16, isOutput=False)
    out = nc.declare_dram_parameter("out", [ROWS_PER_CORE, NODES], bf16, isOutput=True)

    with tile.TileContext(nc) as tc:
        with (
            tc.tile_pool(name="const", bufs=1) as cpool,
            tc.tile_pool(name="psum", bufs=2, space=bass.MemorySpace.PSUM) as pspool,
            tc.tile_pool(name="e", bufs=2) as epool,
            tc.tile_pool(name="m", bufs=2) as mpool,
            tc.tile_pool(name="o", bufs=3) as opool,
            tc.tile_pool(name="stats", bufs=4) as spool,
        ):
            # Operands replicated at partition offsets 0 and GRP so two PE
            # row-groups can run matmuls concurrently.
            a1 = cpool.tile([GRP + KS, ROWS_PER_CORE], bf16)
            nc.sync.dma_start(a1[0:KS, :], n1s[:])
            nc.sync.dma_start(a1[GRP : GRP + KS, :], n1s[:])
            a2 = cpool.tile([GRP + KS, NODES], bf16)
            nc.sync.dma_start(a2[0:KS, :], n2s[:])
            nc.sync.dma_start(a2[GRP : GRP + KS, :], n2s[:])

            for rt in range(N_RT):
                e = epool.tile([RT, NODES], bf16)
                for g in range(NODES // PSUM_COLS):
                    ps = pspool.tile([RT, PSUM_COLS], f32)
                    for c in range(PSUM_COLS // MM_N):
                        col = g * PSUM_COLS + c * MM_N
                        p0 = (c % 2) * GRP  # alternate PE row-groups
                        nc.tensor.matmul(
                            ps[:, c * MM_N : (c + 1) * MM_N],
                            a1[p0 : p0 + KS, rt * RT : (rt + 1) * RT],
                            a2[p0 : p0 + KS, col : col + MM_N],
                            start=True,
                            stop=True,
                        )
                    nc.scalar.activation(
                        e[:, g * PSUM_COLS : (g + 1) * PSUM_COLS], ps[:], AF.Exp
                    )

                m = mpool.tile([RT, NODES], bf16)
                z = spool.tile([RT, 1], f32)
                # m = max(e, 1) == exp(relu(scores)); z = row-sum of m
                nc.vector.tensor_scalar(
                    m[:], e[:], 1.0, None, OP.max, OP.add, accum_out=z[:]
                )
                inv = spool.tile([RT, 1], f32)
                nc.vector.reciprocal(inv[:], z[:])
                o = opool.tile([RT, NODES], bf16)
                H = NODES // 2
                for h in range(2):
                    nc.vector.tensor_scalar(
                        o[:, h * H : (h + 1) * H],
                        m[:, h * H : (h + 1) * H],
                        inv[:],
                        None,
                        OP.mult,
                        OP.bypass,
                    )
                    nc.sync.dma_start(
                        out[rt * RT : (rt + 1) * RT, h * H : (h + 1) * H],
                        o[:, h * H : (h + 1) * H],
                    )

    nc.compile()
    return nc


def kernel(nodevec1: np.ndarray, nodevec2: np.ndarray) -> np.ndarray:
    from concourse.bass_utils import run_bass_kernel_spmd

    global _cached_nc, LAST_RESULTS
    if _cached_nc is None:
        _cached_nc = _build()
    nc = _cached_nc

    bf = ml_dtypes.bfloat16
    n1 = np.asarray(nodevec1, dtype=np.float32)
    n2 = np.asarray(nodevec2, dtype=np.float32)

    h1 = n1.astype(bf)
    l1 = (n1 - h1.astype(np.float32)).astype(bf)
    h2 = n2.astype(bf)
    l2 = (n2 - h2.astype(np.float32)).astype(bf)

    n2s = np.ascontiguousarray(np.concatenate([h2, h2, l2], axis=0))  # [30, 8192]

    in_maps = []
    for i in range(N_CORES):
        sl = slice(i * ROWS_PER_CORE, (i + 1) * ROWS_PER_CORE)
        n1s_i = np.ascontiguousarray(
            np.concatenate([h1[sl].T, l1[sl].T, h1[sl].T], axis=0)
        )  # [30, 1024]
        in_maps.append({"n1s": n1s_i, "n2s": n2s})

    res = run_bass_kernel_spmd(nc, in_maps, core_ids=list(range(N_CORES)))
    LAST_RESULTS = res
    blocks = [
        np.asarray(res.results[i]["out"]).astype(np.float32) for i in range(N_CORES)
    ]
    return np.concatenate(blocks, axis=0)


# revision 3
# speedup vs baseline: 1.0855x; 1.0855x over previous
"""Trainium2 Bass kernel for softmax(relu(nodevec1 @ nodevec2), axis=1).

nodevec1: [8192, 10] f32, nodevec2: [10, 8192] f32 -> out [8192, 8192] f32.

Strategy (8 NeuronCores, no collectives needed):
- Row-shard nodevec1: core i computes rows [i*1024, (i+1)*1024).
- Host-side prep: split each f32 input into bf16 hi+lo pairs and stack
  along the contraction dim (K=30: h1*h2 + l1*h2 + h1*l2), so the PE runs
  at bf16 speed with ~f32 accuracy. Also pre-transpose the nodevec1 shard
  to the [K, M] layout the PE wants for the stationary operand.
- The K=30 operands are loaded twice (SBUF partition offsets 0 and 64) so
  matmuls alternate between two PE row-groups and run pairwise-concurrent
  (tile_position row packing).
- Per 128-row tile: matmul (K=30) -> PSUM, ACT exp: PSUM -> SBUF bf16,
  DVE tensor_scalar max(e,1) with fused row-sum (exp(relu(x)) ==
  max(exp(x),1)), DVE reciprocal, DVE tensor_scalar scale -> bf16 out,
  DMA out. Row softmax is local to each core.
- Output is written bf16 (halves the HBM write) and widened to f32 on the
  host; softmax values are well inside bf16's safe range.
"""

import os

import numpy as np
import ml_dtypes

NODES = 8192
RANK = 10
N_CORES = 8
ROWS_PER_CORE = NODES // N_CORES  # 1024
RT = 128  # rows per tile (SBUF partition dim)
N_RT = ROWS_PER_CORE // RT  # 8
KS = 3 * RANK  # 30: [h1; l1; h1] x [h2; h2; l2]
PSUM_COLS = 2048  # 4 banks per psum tile
MM_N = 512  # one PSUM bank per matmul
GRP = 64  # partition offset of the second PE row-group replica

_cached_nc = None
LAST_RESULTS = None  # BassKernelResults from the most recent run (for test.py)


def _build():
    import concourse.bass as bass
    import concourse.tile as tile
    from concourse import bacc, mybir

    bf16 = mybir.dt.bfloat16
    f32 = mybir.dt.float32
    AF = mybir.ActivationFunctionType
    OP = mybir.AluOpType

    nc = bacc.Bacc(None, target_bir_lowering=False, debug=False)

    n1s = nc.declare_dram_parameter("n1s", [KS, ROWS_PER_CORE], bf16, isOutput=False)
    n2s = nc.declare_dram_parameter("n2s", [KS, NODES], bf16, isOutput=False)
    out = nc.declare_dram_parameter("out", [ROWS_PER_CORE, NODES], bf16, isOutput=True)

    with tile.TileContext(nc) as tc:
        with (
            tc.tile_pool(name="const", bufs=1) as cpool,
            tc.tile_pool(name="psum", bufs=2, space=bass.MemorySpace.PSUM) as pspool,
            tc.tile_pool(name="e", bufs=2) as epool,
            tc.tile_pool(name="m", bufs=2) as mpool,
            tc.tile_pool(name="o", bufs=3) as opool,
            tc.tile_pool(name="stats", bufs=4) as spool,
        ):
            # Operands replicated at partition offsets 0 and GRP so two PE
            # row-groups can run matmuls concurrently.
            a1 = cpool.tile([GRP + KS, ROWS_PER_CORE], bf16)
            nc.sync.dma_start(a1[0:KS, :], n1s[:])
            nc.sync.dma_start(a1[GRP : GRP + KS, :], n1s[:])
            a2 = cpool.tile([GRP + KS, NODES], bf16)
            nc.sync.dma_start(a2[0:KS, :], n2s[:])
            nc.sync.dma_start(a2[GRP : GRP + KS, :], n2s[:])

            for rt in range(N_RT):
                r = mpool.tile([RT, NODES], f32)
                for g in range(NODES // PSUM_COLS):
                    ps = pspool.tile([RT, PSUM_COLS], f32)
                    for c in range(PSUM_COLS // MM_N):
                        col = g * PSUM_COLS + c * MM_N
                        p0 = (c % 2) * GRP  # alternate PE row-groups
                        nc.tensor.matmul(
                            ps[:, c * MM_N : (c + 1) * MM_N],
                            a1[p0 : p0 + KS, rt * RT : (rt + 1) * RT],
                            a2[p0 : p0 + KS, col : col + MM_N],
                            start=True,
                            stop=True,
                        )
                    # drain PSUM with relu; chunk 0 on ACT, rest on DVE to
                    # balance engine load (ACT also runs the big exp pass)
                    rg = r[:, g * PSUM_COLS : (g + 1) * PSUM_COLS]
                    if g == 0:
                        nc.scalar.activation(rg, ps[:], AF.Relu)
                    else:
                        nc.vector.tensor_scalar(
                            rg, ps[:], 0.0, None, OP.max, OP.bypass
                        )

                # e = exp(relu(scores)); z = row-sum rides the ACT pass free
                e = epool.tile([RT, NODES], bf16)
                z = spool.tile([RT, 1], f32)
                nc.scalar.activation(e[:], r[:], AF.Exp, accum_out=z[:])
                inv = spool.tile([RT, 1], f32)
                nc.vector.reciprocal(inv[:], z[:])
                o = opool.tile([RT, NODES], bf16)
                H = NODES // 2
                for h in range(2):
                    nc.vector.tensor_scalar(
                        o[:, h * H : (h + 1) * H],
                        e[:, h * H : (h + 1) * H],
                        inv[:],
                        None,
                        OP.mult,
                        OP.bypass,
                    )
                    nc.sync.dma_start(
                        out[rt * RT : (rt + 1) * RT, h * H : (h + 1) * H],
                        o[:, h * H : (h + 1) * H],
                    )

    nc.compile()
    return nc


def kernel(nodevec1: np.ndarray, nodevec2: np.ndarray) -> np.ndarray:
    from concourse.bass_utils import run_bass_kernel_spmd

    global _cached_nc, LAST_RESULTS
    if _cached_nc is None:
        _cached_nc = _build()
    nc = _cached_nc

    bf = ml_dtypes.bfloat16
    n1 = np.asarray(nodevec1, dtype=np.float32)
    n2 = np.asarray(nodevec2, dtype=np.float32)

    h1 = n1.astype(bf)
    l1 = (n1 - h1.astype(np.float32)).astype(bf)
    h2 = n2.astype(bf)
    l2 = (n2 - h2.astype(np.float32)).astype(bf)

    n2s = np.ascontiguousarray(np.concatenate([h2, h2, l2], axis=0))  # [30, 8192]

    in_maps = []
    for i in range(N_CORES):
        sl = slice(i * ROWS_PER_CORE, (i + 1) * ROWS_PER_CORE)
        n1s_i = np.ascontiguousarray(
            np.concatenate([h1[sl].T, l1[sl].T, h1[sl].T], axis=0)
        )  # [30, 1024]
        in_maps.append({"n1s": n1s_i, "n2s": n2s})

    res = run_bass_kernel_spmd(nc, in_maps, core_ids=list(range(N_CORES)))
    LAST_RESULTS = res
    blocks = [
        np.asarray(res.results[i]["out"]).astype(np.float32) for i in range(N_CORES)
    ]
    return np.concatenate(blocks, axis=0)


# revision 6
# speedup vs baseline: 1.1769x; 1.0842x over previous
"""Trainium2 Bass kernel for softmax(relu(nodevec1 @ nodevec2), axis=1).

nodevec1: [8192, 10] f32, nodevec2: [10, 8192] f32 -> out [8192, 8192] f32.

Strategy (8 NeuronCores, no collectives needed):
- Row-shard nodevec1: core i computes rows [i*1024, (i+1)*1024).
- Host-side prep: split each f32 input into bf16 hi+lo pairs and stack
  along the contraction dim (K=30: h1*h2 + l1*h2 + h1*l2), so the PE runs
  at bf16 speed with ~f32 accuracy. Also pre-transpose the nodevec1 shard
  to the [K, M] layout the PE wants for the stationary operand.
- The K=30 operands are loaded twice (SBUF partition offsets 0 and 64) so
  matmuls alternate between two PE row-groups and run pairwise-concurrent
  (tile_position row packing).
- Per 128-row tile: matmul (K=30) -> PSUM, ACT exp: PSUM -> SBUF bf16,
  DVE tensor_scalar max(e,1) with fused row-sum (exp(relu(x)) ==
  max(exp(x),1)), DVE reciprocal, DVE tensor_scalar scale -> bf16 out,
  DMA out. Row softmax is local to each core.
- Output is written bf16 (halves the HBM write) and widened to f32 on the
  host; softmax values are well inside bf16's safe range.
"""

import os

import numpy as np
import ml_dtypes

NODES = 8192
RANK = 10
N_CORES = 8
ROWS_PER_CORE = NODES // N_CORES  # 1024
RT = 128  # rows per tile (SBUF partition dim)
N_RT = ROWS_PER_CORE // RT  # 8
KS = 3 * RANK  # 30: [h1; l1; h1] x [h2; h2; l2]
PSUM_COLS = 2048  # 4 banks per psum tile
MM_N = 512  # one PSUM bank per matmul
GRP = 64  # partition offset of the second PE row-group replica

_cached_nc = None
LAST_RESULTS = None  # BassKernelResults from the most recent run (for test.py)


def _build():
    import concourse.bass as bass
    import concourse.tile as tile
    from concourse import bacc, mybir

    bf16 = mybir.dt.bfloat16
    f32 = mybir.dt.float32
    AF = mybir.ActivationFunctionType
    OP = mybir.AluOpType

    nc = bacc.Bacc(None, target_bir_lowering=False, debug=False)

    n1s = nc.declare_dram_parameter("n1s", [KS, ROWS_PER_CORE], bf16, isOutput=False)
    n2s = nc.declare_dram_parameter("n2s", [KS, NODES], bf16, isOutput=False)
    out = nc.declare_dram_parameter("out", [ROWS_PER_CORE, NODES], bf16, isOutput=True)

    with tile.TileContext(nc) as tc:
        with (
            tc.tile_pool(name="const", bufs=1) as cpool,
            tc.tile_pool(name="psum", bufs=2, space=bass.MemorySpace.PSUM) as pspool,
            tc.tile_pool(name="e", bufs=2) as epool,
            tc.tile_pool(name="m", bufs=2) as mpool,
            tc.tile_pool(name="o", bufs=3) as opool,
            tc.tile_pool(name="stats", bufs=4) as spool,
        ):
            # Operands replicated at partition offsets 0 and GRP so two PE
            # row-groups can run matmuls concurrently.
            a1 = cpool.tile([GRP + KS, ROWS_PER_CORE], bf16)
            nc.sync.dma_start(a1[0:KS, :], n1s[:])
            nc.sync.dma_start(a1[GRP : GRP + KS, :], n1s[:])
            a2 = cpool.tile([GRP + KS, NODES], bf16)
            # chunked so the first matmuls can start before the whole load
            for ch in range(4):
                cs = slice(ch * (NODES // 4), (ch + 1) * (NODES // 4))
                nc.sync.dma_start(a2[0:KS, cs], n2s[:, cs])
                nc.sync.dma_start(a2[GRP : GRP + KS, cs], n2s[:, cs])

            for rt in range(N_RT):
                r = mpool.tile([RT, NODES], f32)
                for g in range(NODES // PSUM_COLS):
                    ps = pspool.tile([RT, PSUM_COLS], f32)
                    for c in range(PSUM_COLS // MM_N):
                        col = g * PSUM_COLS + c * MM_N
                        p0 = (c % 2) * GRP  # alternate PE row-groups
                        nc.tensor.matmul(
                            ps[:, c * MM_N : (c + 1) * MM_N],
                            a1[p0 : p0 + KS, rt * RT : (rt + 1) * RT],
                            a2[p0 : p0 + KS, col : col + MM_N],
                            start=True,
                            stop=True,
                        )
                    # drain PSUM with relu; the LAST chunk goes to ACT (it
                    # runs right before ACT's exp anyway) so DVE can start
                    # draining chunk 0 as soon as its matmuls land
                    rg = r[:, g * PSUM_COLS : (g + 1) * PSUM_COLS]
                    if g == 3:
                        nc.scalar.activation(rg, ps[:], AF.Relu)
                    else:
                        nc.vector.tensor_scalar(
                            rg, ps[:], 0.0, None, OP.max, OP.bypass
                        )

                # e = exp(relu(scores)); z = row-sum rides the ACT pass free
                e = epool.tile([RT, NODES], bf16)
                z = spool.tile([RT, 1], f32)
                nc.scalar.activation(e[:], r[:], AF.Exp, accum_out=z[:])
                inv = spool.tile([RT, 1], f32)
                nc.vector.reciprocal(inv[:], z[:])
                o = opool.tile([RT, NODES], bf16)
                nh = 4 if rt == N_RT - 1 else 2  # finer pieces shrink the tail
                H = NODES // nh
                for h in range(nh):
                    nc.vector.tensor_scalar(
                        o[:, h * H : (h + 1) * H],
                        e[:, h * H : (h + 1) * H],
                        inv[:],
                        None,
                        OP.mult,
                        OP.bypass,
                    )
                    nc.sync.dma_start(
                        out[rt * RT : (rt + 1) * RT, h * H : (h + 1) * H],
                        o[:, h * H : (h + 1) * H],
                    )

    nc.compile()
    return nc


def kernel(nodevec1: np.ndarray, nodevec2: np.ndarray) -> np.ndarray:
    from concourse.bass_utils import run_bass_kernel_spmd

    global _cached_nc, LAST_RESULTS
    if _cached_nc is None:
        _cached_nc = _build()
    nc = _cached_nc

    bf = ml_dtypes.bfloat16
    n1 = np.asarray(nodevec1, dtype=np.float32)
    n2 = np.asarray(nodevec2, dtype=np.float32)

    h1 = n1.astype(bf)
    l1 = (n1 - h1.astype(np.float32)).astype(bf)
    h2 = n2.astype(bf)
    l2 = (n2 - h2.astype(np.float32)).astype(bf)

    n2s = np.ascontiguousarray(np.concatenate([h2, h2, l2], axis=0))  # [30, 8192]

    in_maps = []
    for i in range(N_CORES):
        sl = slice(i * ROWS_PER_CORE, (i + 1) * ROWS_PER_CORE)
        n1s_i = np.ascontiguousarray(
            np.concatenate([h1[sl].T, l1[sl].T, h1[sl].T], axis=0)
        )  # [30, 1024]
        in_maps.append({"n1s": n1s_i, "n2s": n2s})

    res = run_bass_kernel_spmd(nc, in_maps, core_ids=list(range(N_CORES)))
    LAST_RESULTS = res
    blocks = [
        np.asarray(res.results[i]["out"]).astype(np.float32) for i in range(N_CORES)
    ]
    return np.concatenate(blocks, axis=0)


# revision 8
# speedup vs baseline: 1.1837x; 1.0058x over previous
"""Trainium2 Bass kernel for softmax(relu(nodevec1 @ nodevec2), axis=1).

nodevec1: [8192, 10] f32, nodevec2: [10, 8192] f32 -> out [8192, 8192] f32.

Strategy (8 NeuronCores, no collectives needed):
- Row-shard nodevec1: core i computes rows [i*1024, (i+1)*1024).
- Host-side prep: split each f32 input into bf16 hi+lo pairs and stack
  along the contraction dim (K=30: h1*h2 + l1*h2 + h1*l2), so the PE runs
  at bf16 speed with ~f32 accuracy. Also pre-transpose the nodevec1 shard
  to the [K, M] layout the PE wants for the stationary operand.
- The K=30 operands are loaded twice (SBUF partition offsets 0 and 64) so
  matmuls alternate between two PE row-groups and run pairwise-concurrent
  (tile_position row packing).
- Per 128-row tile: matmul (K=30) -> PSUM, ACT exp: PSUM -> SBUF bf16,
  DVE tensor_scalar max(e,1) with fused row-sum (exp(relu(x)) ==
  max(exp(x),1)), DVE reciprocal, DVE tensor_scalar scale -> bf16 out,
  DMA out. Row softmax is local to each core.
- Output is written bf16 (halves the HBM write) and widened to f32 on the
  host; softmax values are well inside bf16's safe range.
"""

import os

import numpy as np
import ml_dtypes

NODES = 8192
RANK = 10
N_CORES = 8
ROWS_PER_CORE = NODES // N_CORES  # 1024
RT = 128  # rows per tile (SBUF partition dim)
N_RT = ROWS_PER_CORE // RT  # 8
KS = 3 * RANK  # 30: [h1; l1; h1] x [h2; h2; l2]
PSUM_COLS = 2048  # 4 banks per psum tile
MM_N = 512  # one PSUM bank per matmul
GRP = 64  # partition offset of the second PE row-group replica

_cached_nc = None
LAST_RESULTS = None  # BassKernelResults from the most recent run (for test.py)


def _build():
    import concourse.bass as bass
    import concourse.tile as tile
    from concourse import bacc, mybir

    bf16 = mybir.dt.bfloat16
    f32 = mybir.dt.float32
    AF = mybir.ActivationFunctionType
    OP = mybir.AluOpType

    nc = bacc.Bacc(None, target_bir_lowering=False, debug=False)

    n1s = nc.declare_dram_parameter("n1s", [KS, ROWS_PER_CORE], bf16, isOutput=False)
    n2s = nc.declare_dram_parameter("n2s", [KS, NODES], bf16, isOutput=False)
    out = nc.declare_dram_parameter("out", [ROWS_PER_CORE, NODES], bf16, isOutput=True)

    with tile.TileContext(nc) as tc:
        with (
            tc.tile_pool(name="const", bufs=1) as cpool,
            tc.tile_pool(name="psum", bufs=2, space=bass.MemorySpace.PSUM) as pspool,
            tc.tile_pool(name="e", bufs=2) as epool,
            tc.tile_pool(name="m", bufs=2) as mpool,
            tc.tile_pool(name="o", bufs=3) as opool,
            tc.tile_pool(name="stats", bufs=4) as spool,
        ):
            # Operands replicated at partition offsets 0 and GRP so two PE
            # row-groups can run matmuls concurrently.
            a1 = cpool.tile([GRP + KS, ROWS_PER_CORE], bf16)
            a2 = cpool.tile([GRP + KS, NODES], bf16)
            # ordered + chunked so the first psum group's operands land first
            nc.sync.dma_start(a1[0:KS, :], n1s[:])
            nc.sync.dma_start(a2[0:KS, 0:PSUM_COLS], n2s[:, 0:PSUM_COLS])
            nc.sync.dma_start(a1[GRP : GRP + KS, :], n1s[:])
            nc.sync.dma_start(
                a2[GRP : GRP + KS, 0:PSUM_COLS], n2s[:, 0:PSUM_COLS]
            )
            nc.sync.dma_start(a2[0:KS, PSUM_COLS:], n2s[:, PSUM_COLS:])
            nc.sync.dma_start(a2[GRP : GRP + KS, PSUM_COLS:], n2s[:, PSUM_COLS:])

            for rt in range(N_RT):
                r = mpool.tile([RT, NODES], f32)
                for g in range(NODES // PSUM_COLS):
                    ps = pspool.tile([RT, PSUM_COLS], f32)
                    for c in range(PSUM_COLS // MM_N):
                        col = g * PSUM_COLS + c * MM_N
                        p0 = (c % 2) * GRP  # alternate PE row-groups
                        nc.tensor.matmul(
                            ps[:, c * MM_N : (c + 1) * MM_N],
                            a1[p0 : p0 + KS, rt * RT : (rt + 1) * RT],
                            a2[p0 : p0 + KS, col : col + MM_N],
                            start=True,
                            stop=True,
                        )
                    # drain PSUM with relu; the LAST chunk goes to ACT (it
                    # runs right before ACT's exp anyway) so DVE can start
                    # draining chunk 0 as soon as its matmuls land
                    rg = r[:, g * PSUM_COLS : (g + 1) * PSUM_COLS]
                    if g == 3:
                        nc.scalar.activation(rg, ps[:], AF.Relu)
                    else:
                        nc.vector.tensor_scalar(
                            rg, ps[:], 0.0, None, OP.max, OP.bypass
                        )

                # e = exp(relu(scores)); z = row-sum rides the ACT pass free
                e = epool.tile([RT, NODES], bf16)
                z = spool.tile([RT, 1], f32)
                nc.scalar.activation(e[:], r[:], AF.Exp, accum_out=z[:])
                inv = spool.tile([RT, 1], f32)
                nc.vector.reciprocal(inv[:], z[:])
                o = opool.tile([RT, NODES], bf16)
                nh = 4 if rt == N_RT - 1 else 2  # finer pieces shrink the tail
                H = NODES // nh
                for h in range(nh):
                    nc.vector.tensor_scalar(
                        o[:, h * H : (h + 1) * H],
                        e[:, h * H : (h + 1) * H],
                        inv[:],
                        None,
                        OP.mult,
                        OP.bypass,
                    )
                    nc.sync.dma_start(
                        out[rt * RT : (rt + 1) * RT, h * H : (h + 1) * H],
                        o[:, h * H : (h + 1) * H],
                    )

    nc.compile()
    return nc


def kernel(nodevec1: np.ndarray, nodevec2: np.ndarray) -> np.ndarray:
    from concourse.bass_utils import run_bass_kernel_spmd

    global _cached_nc, LAST_RESULTS
    if _cached_nc is None:
        _cached_nc = _build()
    nc = _cached_nc

    bf = ml_dtypes.bfloat16
    n1 = np.asarray(nodevec1, dtype=np.float32)
    n2 = np.asarray(nodevec2, dtype=np.float32)

    h1 = n1.astype(bf)
    l1 = (n1 - h1.astype(np.float32)).astype(bf)
    h2 = n2.astype(bf)
    l2 = (n2 - h2.astype(np.float32)).astype(bf)

    n2s = np.ascontiguousarray(np.concatenate([h2, h2, l2], axis=0))  # [30, 8192]

    in_maps = []
    for i in range(N_CORES):
        sl = slice(i * ROWS_PER_CORE, (i + 1) * ROWS_PER_CORE)
        n1s_i = np.ascontiguousarray(
            np.concatenate([h1[sl].T, l1[sl].T, h1[sl].T], axis=0)
        )  # [30, 1024]
        in_maps.append({"n1s": n1s_i, "n2s": n2s})

    res = run_bass_kernel_spmd(nc, in_maps, core_ids=list(range(N_CORES)))
    LAST_RESULTS = res
    blocks = [
        np.asarray(res.results[i]["out"]).astype(np.float32) for i in range(N_CORES)
    ]
    return np.concatenate(blocks, axis=0)
